# revision 6
# baseline (speedup 1.0000x reference)
"""DSPA (dual-softmax sparse attention) Trainium2 kernel.

Math (reference):
  x1 = x.reshape(2, 64, 4096)                       # [b, c, n]
  x2 = dilated_conv1d(x1, w, b) (k=3, dil=2, pad=1) # [b, c, m], m = n-2
  s[b] = x1[b]^T @ x2[b]                            # [n, m]
  x3 = softmax over b (2 elements)  ->  x3[0] = sigmoid(s0 - s1), x3[1] = 1 - x3[0]
  x4[b] = x2[b] @ x3[b]^T                           # [c, n]
  out = x1 + x4

Key identities used:
  p := sigmoid(d),  d := s0 - s1
  x4[0] = x2[0] @ p^T
  x4[1] = S1 - x2[1] @ p^T          with S1[c] = sum_j x2[1][c, j]

Device strategy (8 cores, no collectives):
  - Shard the query dim n=4096 -> 512 columns per core.
  - Each core computes the full conv x2 on-chip in a batch-stacked,
    batch1-NEGATED bf16 layout x2bf [128, 4096] so that ONE matmul with
    K=128 (c of batch0 stacked on c of batch1) yields d^T directly in PSUM.
  - x2^T tiles come from per-chunk DMA XBAR transposes (one 3-D-AP
    instruction per 512-col chunk transposes its four 128x128 blocks
    in place), freeing the PE of 32 transpose matmuls and the DVE of the
    PSUM evacuation copies.
  - d^T groups are 3 j-tiles wide ([128, 1536] fp32 = 3 PSUM banks,
    double-buffered; psum_conv drops to 1 buffer to fit 8 banks), so one
    ACT sigmoid call covers 3 tiles (12 calls instead of 18 - the ACT
    fixed cost per call is ~290 ns).
  - Input rides both HWDGE rings: scalar carries only the conv weights
    (the scalar ENGINE is nearly saturated by sigmoids); sync carries x1
    chunks with the conv-critical piece first, and the later input chunks
    are issued mid-schedule so the x2 transposes interleave at the right
    ring position.
  - x4 matmuls consume p tiles one group LATE so the sigmoid->matmul
    dependency never stalls the PE; the final group runs per-tile
    sigmoid->matmul pairs to shorten the tail chain.

Everything is bf16 except the PSUM x4/conv accumulations (fp32) and the
final epilogue; matmul weight loads get the FWL fast path and hide in the
PE's background weight buffer.
"""

import numpy as np

import concourse.bacc as bacc
import concourse.mybir as mybir
import concourse.tile as tile
from concourse.bass_utils import run_bass_kernel_spmd

F32 = mybir.dt.float32
BF16 = mybir.dt.bfloat16

B, C, N, M = 2, 64, 4096, 4094
NCORES = 8
ISL = N // NCORES          # 512 query columns per core
NT = 32                    # j tiles of 128 (last = 126)
NCH = 8                    # conv chunks of 512 (last = 510)
MISC_W = 3 * 128 + 2 + ISL  # weights + 2 bf16 cols hold fp32 bias bits + x1q
NWARM = 4                  # bf16 N=512 warm matmuls (cold ~427ns each)


def build_nc():
    nc = bacc.Bacc()

    xin = nc.dram_tensor("xin", [128, N + 2], BF16, kind="ExternalInput")
    misc = nc.dram_tensor("misc", [128, MISC_W], BF16, kind="ExternalInput")
    out = nc.dram_tensor("out", [128, ISL], F32, kind="ExternalOutput")

    with tile.TileContext(nc) as tc:
        with (
            tc.tile_pool(name="persist", bufs=1) as persist,
            tc.tile_pool(name="ptiles", bufs=3) as ptiles,
            tc.tile_pool(name="psum_acc", bufs=1, space="PSUM") as psum_acc,
        ):
            x1pads = persist.tile([128, N + 2], BF16)
            msb = persist.tile([128, MISC_W], BF16)
            x2bf = persist.tile([128, NT * 128], BF16)
            x2T2bf = persist.tile([128, NT * 128], BF16)
            scratch = persist.tile([128, 1], F32)
            warm_sb = persist.tile([128, 256], F32)
            s1corr = persist.tile([128, 1], F32)
            s1parts = persist.tile([128, 2 * NCH], F32)
            osb = persist.tile([128, ISL], F32)

            def wsb(k):
                return msb[:, 128 * k : 128 * (k + 1)]

            bsb = msb[:, 384:386].bitcast(F32)
            x1q = msb[:, 386 : 386 + ISL]

            # Warm-train scratch on the GPSIMD queue (free earliest; the
            # framework const memsets precede it and finish by ~6.1us).
            nc.gpsimd.memset(warm_sb[:, :], 0.0)

            # Input loads. xin arrives host-padded ([0, x1, 0]).
            # scalar ring: ONLY the conv weights (99KB) - the scalar engine
            # is nearly saturated by sigmoids later, so everything else
            # stays off it. sync ring: x1 chunks in need order; the small
            # first chunk's completion fires early so conv starts sooner.
            # Later chunks (c2, c3) are issued mid-schedule (inside
            # conv_chunk) so the x2 transposes take the right ring slot.
            CUTS = [0, 520, 1720, 2920, N + 2]
            nc.scalar.dma_start(msb[:, 0:386], misc[:, 0:386])
            nc.sync.dma_start(x1pads[:, CUTS[0] : CUTS[1]], xin[:, CUTS[0] : CUTS[1]])
            nc.sync.dma_start(msb[:, 386:MISC_W], misc[:, 386:MISC_W])
            nc.sync.dma_start(x1pads[:, CUTS[1] : CUTS[2]], xin[:, CUTS[1] : CUTS[2]])

            # Preload the sigmoid ACT table set while DMAs run (scalar
            # engine: w-issue, then tables - done well before sigmoid g0).
            nc.vector.memset(scratch[:, :], 0.0)
            nc.scalar.activation(
                scratch[:, :], scratch[:, :], mybir.ActivationFunctionType.Sigmoid
            )
            nc.vector.memset(x2bf[:, M : NT * 128], 0.0)

            # Warm the PE (HAM clock gate) while the input DMAs run: bf16
            # zero matmuls sized to end right at data-ready (~8.6us).
            with tc.tile_pool(name="psum_warm", bufs=1, space="PSUM") as psum_warm:
                wps = psum_warm.tile([128, 512], F32)
                wbf = warm_sb[:, :].bitcast(BF16)
                for _ in range(NWARM):
                    nc.tensor.matmul(wps[:, :], wbf[:, 0:128], wbf[:, 0:512])
                for _ in range(2):
                    nc.tensor.matmul(wps[:, 0:128], wbf[:, 0:128], wbf[:, 0:128])

            acc = psum_acc.tile([128, ISL], F32)
            with (
                tc.tile_pool(name="psum_conv", bufs=1, space="PSUM") as psum_conv,
                tc.tile_pool(name="psum_d", bufs=2, space="PSUM") as psum_d,
            ):

                def conv_chunk(ch):
                    j0 = ch * 512
                    w = min(512, M - j0)
                    pc = psum_conv.tile([128, 512], F32, name=f"pc{ch}", tag="pc")
                    for k in range(3):
                        nc.tensor.matmul(
                            pc[:, 0:w],
                            wsb(k),
                            x1pads[:, j0 + 2 * k : j0 + 2 * k + w],
                            start=(k == 0),
                            stop=(k == 2),
                        )
                    if ch == 0:
                        # Ramp fillers: everything after c0's matmuls waits
                        # on the DVE evacuation; these bf16 matmuls on the
                        # already-loaded chunk-0 data (into acc, which the
                        # first x4 matmul's start=True clears later) bridge
                        # that wait so the HAM clock never disengages.
                        for _ in range(2):
                            nc.tensor.matmul(
                                acc[:, :],
                                x1pads[:, 0:128],
                                x1pads[:, 0:512],
                                start=True,
                                stop=True,
                                skip_group_check=True,
                            )
                    # Evacuate in 256-col halves so the d-matmuls unblock
                    # at half-chunk granularity; the per-chunk row sums
                    # (for the S1 correction) come for free.
                    for h in range(2):
                        e0, e1 = 256 * h, min(256 * (h + 1), w)
                        nc.vector.tensor_scalar(
                            x2bf[:, j0 + e0 : j0 + e1],
                            pc[:, e0:e1],
                            bsb,
                            0.0,
                            op0=mybir.AluOpType.add,
                            op1=mybir.AluOpType.add,
                            accum_out=s1parts[:, 2 * ch + h : 2 * ch + h + 1],
                        )
                    # Blockwise x2^T via one DMA XBAR transpose: the 3-D
                    # out AP [p, block, 128] makes the engine transpose
                    # each 128x128 block in place (out[:,e,:] = in-blk-e^T).
                    nc.sync.dma_start(
                        x2T2bf[:, j0 : j0 + 512].rearrange("p (b l) -> p b l", l=128),
                        x2bf[:, j0 : j0 + 512],
                        transpose=True,
                    )
                    # Mid-schedule input issues: keeps the sync ring FIFO in
                    # need order (c0, x1q, c1, tr0, c2, tr1, c3, tr2, ...).
                    if ch == 0:
                        nc.sync.dma_start(
                            x1pads[:, CUTS[2] : CUTS[3]], xin[:, CUTS[2] : CUTS[3]]
                        )
                    elif ch == 1:
                        nc.sync.dma_start(
                            x1pads[:, CUTS[3] : CUTS[4]], xin[:, CUTS[3] : CUTS[4]]
                        )

                def mm2(p, ts):
                    for u, t in enumerate(ts):
                        jt = t * 128
                        # K=128 always: tile 31's two missing j rows are
                        # zero in x2T2bf, so stale p rows contribute 0.
                        nc.tensor.matmul(
                            acc[:, :],
                            x2T2bf[:, jt : jt + 128],
                            p[:, 512 * u : 512 * u + 512],
                            start=(t == 0),
                            stop=(t == NT - 1),
                        )

                # x4 matmuls consume p tiles one group LATE: by emission
                # time the sigmoid is long finished, so the mm2 never pays
                # the first-matmul-after-semaphore SBUF-latency tax.
                pending = []

                def group(gg, ts, split=False):
                    # d^T tiles: up to 3 j-tiles = [128, 1536] fp32 =
                    # 3 PSUM banks, ONE sigmoid call per group.
                    gw = 512 * len(ts)
                    d = psum_d.tile([128, 1536], F32, name=f"d{gg}", tag="d")
                    for u, t in enumerate(ts):
                        jt = t * 128
                        wt = min(128, M - jt)
                        nc.tensor.matmul(
                            d[0:wt, 512 * u : 512 * u + 512],
                            x2bf[:, jt : jt + wt],
                            x1q,
                            start=True,
                            stop=True,
                        )
                    p = ptiles.tile([128, 1536], BF16, name=f"p{gg}", tag="p")
                    if not split:
                        nc.scalar.activation(
                            p[:, 0:gw],
                            d[:, 0:gw],
                            mybir.ActivationFunctionType.Sigmoid,
                        )
                        while pending:
                            mm2(*pending.pop(0))
                        pending.append((p, ts))
                        return
                    # Final group: flush the lagged matmuls, then per-tile
                    # sigmoids interleaved with the x4 matmuls keep the very
                    # last sigmoid->matmul dependency chain short.
                    while pending:
                        mm2(*pending.pop(0))
                    for u, t in enumerate(ts):
                        nc.scalar.activation(
                            p[:, 512 * u : 512 * u + 512],
                            d[:, 512 * u : 512 * u + 512],
                            mybir.ActivationFunctionType.Sigmoid,
                        )
                        nc.tensor.matmul(
                            acc[:, :],
                            x2T2bf[:, t * 128 : t * 128 + 128],
                            p[:, 512 * u : 512 * u + 512],
                            start=(t == 0),
                            stop=(t == NT - 1),
                        )

                # Groups of 3 j-tiles (g0..g9) + a final split pair (g10).
                # Conv chunk ch unlocks tiles 4ch..4ch+3; each group g needs
                # tiles 3g..3g+2, so chunks are issued just-in-time.
                GROUPS = [[3 * g + i for i in range(3)] for g in range(10)]
                GROUPS.append([30, 31])
                SCHED = [
                    ("c", 0), ("g", 0),
                    ("c", 1), ("g", 1),
                    ("c", 2), ("g", 2), ("g", 3),
                    ("c", 3), ("g", 4),
                    ("c", 4), ("g", 5),
                    ("c", 5), ("g", 6), ("g", 7),
                    ("c", 6), ("g", 8),
                    ("c", 7), ("g", 9), ("s1", 0), ("gs", 10),
                ]
                for kind, idx in SCHED:
                    if kind == "c":
                        conv_chunk(idx)
                    elif kind == "g":
                        group(idx, GROUPS[idx])
                    elif kind == "gs":
                        group(idx, GROUPS[idx], split=True)
                    else:
                        # S1 correction vector: zeros on top, +S1[c] on the
                        # bottom (negated sum of -x2[1] partial row sums).
                        nc.vector.memset(s1corr[0:64, 0:1], 0.0)
                        nc.vector.reduce_sum(
                            s1corr[64:128, 0:1],
                            s1parts[64:128, 0 : 2 * NCH],
                            axis=mybir.AxisListType.X,
                            negate=True,
                        )

            # Epilogue: out = (acc + S1corr) + x1 in DVE passes (acc bottom
            # holds -x2[1] @ p^T, so adding S1 gives x4[1]).  The two
            # halves go out on DIFFERENT rings so issue+transfer overlap.
            HALF = ISL // 2
            for h in range(2):
                sl = slice(h * HALF, (h + 1) * HALF)
                nc.vector.scalar_tensor_tensor(
                    osb[:, sl],
                    acc[:, sl],
                    s1corr[:, 0:1],
                    x1q[:, sl],
                    op0=mybir.AluOpType.add,
                    op1=mybir.AluOpType.add,
                )
                if h == 0:
                    nc.scalar.dma_start(out[:, sl], osb[:, sl])
                else:
                    nc.sync.dma_start(out[:, sl], osb[:, sl])

    nc.finalize()
    return nc


_NC_CACHE = None


def _get_nc():
    global _NC_CACHE
    if _NC_CACHE is None:
        _NC_CACHE = build_nc()
    return _NC_CACHE


def _host_prep(x, conv_w, conv_b):
    import ml_dtypes

    x1 = np.zeros((B * C, N + 2), dtype=np.float32)
    x1[:, 1 : N + 1] = x.reshape(B * C, N)
    x1 = x1.astype(ml_dtypes.bfloat16)
    misc = np.zeros((128, MISC_W), dtype=np.float32)
    for k in range(3):
        wT = conv_w[:, :, k].T.astype(np.float32)  # [i, o]
        misc[0:64, 128 * k : 128 * k + 64] = wT
        misc[64:128, 128 * k + 64 : 128 * k + 128] = -wT
    misc = misc.astype(ml_dtypes.bfloat16)
    bias = np.concatenate([conv_b, -conv_b]).astype(np.float32).reshape(128, 1)
    misc[:, 384:386] = bias.view(np.uint32).view(ml_dtypes.bfloat16).reshape(128, 2)
    return x1, misc


def kernel(x, conv_w, conv_b, _trace=False):
    x = np.asarray(x)
    conv_w = np.asarray(conv_w)
    conv_b = np.asarray(conv_b)
    x1, misc = _host_prep(x, conv_w, conv_b)

    in_maps = []
    for r in range(NCORES):
        mr = misc.copy()
        mr[:, 386 : 386 + ISL] = x1[:, 1 + r * ISL : 1 + (r + 1) * ISL]
        in_maps.append({"xin": x1, "misc": mr})

    nc = _get_nc()
    res = run_bass_kernel_spmd(nc, in_maps, list(range(NCORES)), trace=_trace)
    out = np.concatenate([res.results[r]["out"] for r in range(NCORES)], axis=1)
    out = out.reshape(B, C, 16, 16, 16).astype(np.float32)
    if _trace:
        return out, res
    return out
